# revision 1
# baseline (speedup 1.0000x reference)
"""DCAlign kernel: data-parallel over batch B=8 across 8 NeuronCores.

Each core processes one sample (per the sharding hint: gathers and convs
are per-sample; small weights replicated). The batch-coupling in the
reference (guide = |x2r[:4] - x2r[4:]| repeated twice) is resolved on the
host by handing core i both x2[i % 4] and x2[i % 4 + 4].
"""

import numpy as np
import jax
import jax.numpy as jnp

K = 3
EPS = 1e-5

# Hardcoded problem shapes (grading contract: no sibling file reads).
B, C, H, W = 8, 256, 64, 64


def _bilinear_sample(img, py, px):
    """Bilinear sample img (H,W,C) at float pixel coords py,px (H,W), zero pad."""
    Hh, Ww, Cc = img.shape
    y0f = jnp.floor(py)
    x0f = jnp.floor(px)
    wy = py - y0f
    wx = px - x0f
    y0 = y0f.astype(jnp.int32)
    x0 = x0f.astype(jnp.int32)
    out = jnp.zeros(img.shape, img.dtype)
    corners = ((0, 0, (1 - wy) * (1 - wx)), (0, 1, (1 - wy) * wx),
               (1, 0, wy * (1 - wx)), (1, 1, wy * wx))
    for dy, dx, w in corners:
        yy = y0 + dy
        xx = x0 + dx
        valid = ((yy >= 0) & (yy < Hh) & (xx >= 0) & (xx < Ww)).astype(img.dtype)
        yc = jnp.clip(yy, 0, Hh - 1)
        xc = jnp.clip(xx, 0, Ww - 1)
        out = out + (w * valid)[..., None] * img[yc, xc]
    return out


def _forward_one(x1, x2a, x2b, dw_w, dw_b, ln_g, ln_b, off_w, off_b,
                 mask_w, mask_b, in_w, in_b, out_w, out_b,
                 bn_g, bn_b, bn_mean, bn_var):
    """Per-sample forward. x1: (C,H,W); x2a/x2b: (C,H/2,W/2)."""
    # bilinear upsample both halves (half-pixel centers == jax.image.resize linear)
    x2ra = jax.image.resize(x2a, (C, H, W), method="linear")
    x2rb = jax.image.resize(x2b, (C, H, W), method="linear")
    guide = jnp.abs(x2ra - x2rb)

    v = jnp.transpose(x1, (1, 2, 0))        # (H,W,C)
    f = jnp.transpose(guide, (1, 2, 0))     # (H,W,C)

    x_proj = v @ in_w + in_b

    feat = jax.lax.conv_general_dilated(
        f[None], dw_w, (1, 1), "SAME",
        dimension_numbers=("NHWC", "HWIO", "NHWC"),
        feature_group_count=C)[0] + dw_b
    mu = jnp.mean(feat, axis=-1, keepdims=True)
    var = jnp.var(feat, axis=-1, keepdims=True)
    feat = (feat - mu) * jax.lax.rsqrt(var + EPS) * ln_g + ln_b
    feat = jax.nn.gelu(feat)

    offset = (feat @ off_w + off_b).reshape(H, W, K * K, 2)
    mask = jax.nn.softmax(feat @ mask_w + mask_b, axis=-1)

    base_y = jnp.arange(H, dtype=x1.dtype)[:, None]
    base_x = jnp.arange(W, dtype=x1.dtype)[None, :]
    acc = jnp.zeros((H, W, C), x1.dtype)
    for k in range(K * K):
        ky = k // K - 1
        kx = k % K - 1
        py = base_y + ky + offset[..., k, 1]
        px = base_x + kx + offset[..., k, 0]
        acc = acc + mask[..., k:k + 1] * _bilinear_sample(x_proj, py, px)

    y = acc @ out_w + out_b
    y = jnp.transpose(y, (2, 0, 1))          # (C,H,W)

    scale = bn_g * jax.lax.rsqrt(bn_var + EPS)
    y = y * scale[:, None, None] + (bn_b - bn_mean * scale)[:, None, None]
    return jax.nn.relu(y) + x1


_N_WEIGHTS = 16  # dw_w .. bn_var, all replicated


def _run_pmap(devices, x1, x2a, x2b, weights):
    fn = jax.pmap(
        _forward_one,
        in_axes=(0, 0, 0) + (None,) * _N_WEIGHTS,
        devices=devices,
    )
    return fn(x1, x2a, x2b, *weights)


def kernel(x1, x2, dw_w, dw_b, ln_g, ln_b, off_w, off_b, mask_w, mask_b,
           in_w, in_b, out_w, out_b, bn_g, bn_b, bn_mean, bn_var):
    x1 = np.asarray(x1, np.float32)
    x2 = np.asarray(x2, np.float32)
    # guide[i] = |resize(x2)[i % 4] - resize(x2)[i % 4 + 4]| -> give core i both samples
    lo = x2[[0, 1, 2, 3, 0, 1, 2, 3]]
    hi = x2[[4, 5, 6, 7, 4, 5, 6, 7]]
    weights = tuple(np.asarray(w, np.float32) for w in
                    (dw_w, dw_b, ln_g, ln_b, off_w, off_b, mask_w, mask_b,
                     in_w, in_b, out_w, out_b, bn_g, bn_b, bn_mean, bn_var))

    devs = jax.devices()
    try:
        if len(devs) >= 8:
            out = _run_pmap(devs[:8], x1, lo, hi, weights)
        else:
            raise RuntimeError(f"only {len(devs)} devices")
    except Exception:
        # fallback: vectorized single-device path (correctness safety net)
        out = jax.vmap(
            _forward_one, in_axes=(0, 0, 0) + (None,) * _N_WEIGHTS
        )(x1, lo, hi, *weights)
    return np.asarray(out, np.float32)


# revision 2
# speedup vs baseline: 2.4308x; 2.4308x over previous
"""DCAlign kernel: data-parallel over batch B=8 across 8 NeuronCores.

Each core processes one sample (per the sharding hint: gathers and convs
are per-sample; small weights replicated). The batch-coupling in the
reference (guide = |x2r[:4] - x2r[4:]| repeated twice) is resolved on the
host by handing core i both x2[i % 4] and x2[i % 4 + 4].
"""

import numpy as np
import jax
import jax.numpy as jnp

K = 3
EPS = 1e-5

# Hardcoded problem shapes (grading contract: no sibling file reads).
B, C, H, W = 8, 256, 64, 64


def _bilinear_sample(img, py, px):
    """Bilinear sample img (H,W,C) at float pixel coords py,px (H,W), zero pad."""
    Hh, Ww, Cc = img.shape
    y0f = jnp.floor(py)
    x0f = jnp.floor(px)
    wy = py - y0f
    wx = px - x0f
    y0 = y0f.astype(jnp.int32)
    x0 = x0f.astype(jnp.int32)
    out = jnp.zeros(img.shape, img.dtype)
    corners = ((0, 0, (1 - wy) * (1 - wx)), (0, 1, (1 - wy) * wx),
               (1, 0, wy * (1 - wx)), (1, 1, wy * wx))
    for dy, dx, w in corners:
        yy = y0 + dy
        xx = x0 + dx
        valid = ((yy >= 0) & (yy < Hh) & (xx >= 0) & (xx < Ww)).astype(img.dtype)
        yc = jnp.clip(yy, 0, Hh - 1)
        xc = jnp.clip(xx, 0, Ww - 1)
        out = out + (w * valid)[..., None] * img[yc, xc]
    return out


def _forward_one(x1, x2a, x2b, dw_w, dw_b, ln_g, ln_b, off_w, off_b,
                 mask_w, mask_b, in_w, in_b, out_w, out_b,
                 bn_g, bn_b, bn_mean, bn_var):
    """Per-sample forward. x1: (C,H,W); x2a/x2b: (C,H/2,W/2)."""
    # bilinear upsample both halves (half-pixel centers == jax.image.resize linear)
    x2ra = jax.image.resize(x2a, (C, H, W), method="linear")
    x2rb = jax.image.resize(x2b, (C, H, W), method="linear")
    guide = jnp.abs(x2ra - x2rb)

    v = jnp.transpose(x1, (1, 2, 0))        # (H,W,C)
    f = jnp.transpose(guide, (1, 2, 0))     # (H,W,C)

    x_proj = v @ in_w + in_b

    feat = jax.lax.conv_general_dilated(
        f[None], dw_w, (1, 1), "SAME",
        dimension_numbers=("NHWC", "HWIO", "NHWC"),
        feature_group_count=C)[0] + dw_b
    mu = jnp.mean(feat, axis=-1, keepdims=True)
    var = jnp.var(feat, axis=-1, keepdims=True)
    feat = (feat - mu) * jax.lax.rsqrt(var + EPS) * ln_g + ln_b
    feat = jax.nn.gelu(feat)

    offset = (feat @ off_w + off_b).reshape(H, W, K * K, 2)
    mask = jax.nn.softmax(feat @ mask_w + mask_b, axis=-1)

    base_y = jnp.arange(H, dtype=x1.dtype)[:, None]
    base_x = jnp.arange(W, dtype=x1.dtype)[None, :]
    acc = jnp.zeros((H, W, C), x1.dtype)
    for k in range(K * K):
        ky = k // K - 1
        kx = k % K - 1
        py = base_y + ky + offset[..., k, 1]
        px = base_x + kx + offset[..., k, 0]
        acc = acc + mask[..., k:k + 1] * _bilinear_sample(x_proj, py, px)

    y = acc @ out_w + out_b
    y = jnp.transpose(y, (2, 0, 1))          # (C,H,W)

    scale = bn_g * jax.lax.rsqrt(bn_var + EPS)
    y = y * scale[:, None, None] + (bn_b - bn_mean * scale)[:, None, None]
    return jax.nn.relu(y) + x1


_N_WEIGHTS = 16  # dw_w .. bn_var, all replicated
_PMAP_CACHE = {}


def _run_pmap(devices, x1, x2a, x2b, weights):
    key = tuple(id(d) for d in devices)
    if key not in _PMAP_CACHE:
        _PMAP_CACHE[key] = jax.pmap(
            _forward_one,
            in_axes=(0, 0, 0) + (None,) * _N_WEIGHTS,
            devices=devices,
        )
    return _PMAP_CACHE[key](x1, x2a, x2b, *weights)


def kernel(x1, x2, dw_w, dw_b, ln_g, ln_b, off_w, off_b, mask_w, mask_b,
           in_w, in_b, out_w, out_b, bn_g, bn_b, bn_mean, bn_var):
    x1 = np.asarray(x1, np.float32)
    x2 = np.asarray(x2, np.float32)
    # guide[i] = |resize(x2)[i % 4] - resize(x2)[i % 4 + 4]| -> give core i both samples
    lo = x2[[0, 1, 2, 3, 0, 1, 2, 3]]
    hi = x2[[4, 5, 6, 7, 4, 5, 6, 7]]
    weights = tuple(np.asarray(w, np.float32) for w in
                    (dw_w, dw_b, ln_g, ln_b, off_w, off_b, mask_w, mask_b,
                     in_w, in_b, out_w, out_b, bn_g, bn_b, bn_mean, bn_var))

    devs = jax.devices()
    try:
        if len(devs) >= 8:
            out = _run_pmap(devs[:8], x1, lo, hi, weights)
        else:
            raise RuntimeError(f"only {len(devs)} devices")
    except Exception:
        # fallback: vectorized single-device path (correctness safety net)
        out = jax.vmap(
            _forward_one, in_axes=(0, 0, 0) + (None,) * _N_WEIGHTS
        )(x1, lo, hi, *weights)
    return np.asarray(out, np.float32)
